# revision 19
# baseline (speedup 1.0000x reference)
"""NetVLAD layer on 8 Trainium2 NeuronCores (Bass/Tile), fp8 DoubleRow.

Problem: descriptors [B=16, D=512, N=4096] f32, W [K=64, D], b [K],
centers [D, K].
  scores = softmax_K(W @ desc + b)            [B, K, N]
  agg[b,d,k] = sum_n scores[b,k,n] desc[b,d,n]
  vlad = agg - centers * sum_n(scores);  intra-L2-norm over D; global L2.

Sharding: data-parallel over B across 8 cores (2 items per core);
W/b/centers replicated.

Host prep: desc quantized to fp8e4m3 ONCE, shipped in BOTH layouts:
  desc8  [B_PER, 128, 4, N]    (p,t,n) = q(desc[i, 128t+p, n])   - mm1 rhs
  descT  [B_PER, 128, 32, 520] (p,c,j) = q(desc[i, j, 128c+p]),
         col 512 = 1.0 (ssum rides mm2), 513..519 pad            - mm2 rhs
  wt     [128, 4, 64] = q(16*W[k, 128t+p])  (exp undoes the 16 via scale)

Per-core kernel (per item):
  - mm1 (fp8 DoubleRow, 2x rate): scores[64, 512-chunk] psum, 2 MMs/chunk
  - ACT: exp_s = Exp(scores/16 + b) -> bf16 SBUF [64, N]
  - expT+Z in one matmul per 128-chunk: lhsT=exp chunk [64,128],
    rhs=[eye64|ones] [64,65] -> psum [128n, 64k | Z]
  - DVE: rz = 1/Z (batched x4); softT = expT * rz -> fp8 (bcast mul)
  - mm2 (fp8 DoubleRow): agg[64k, 256|257] += softT_pair.T @ descT_pair
    col 512 of descT = ones => agg[:,512] = ssum
  - tail: vlad = cneg*ssum + agg; intra-norm over D (free dim);
    global norm via ones-matmul partition reduction; DMA out [64, 512]
Host side: concat over B, transpose [K, D] -> [D, K] flatten.
"""

import sys

sys.path.insert(0, "/opt/trn_rl_repo")

import numpy as np
import ml_dtypes

B, D, K, N = 16, 512, 64, 4096
N_CORES = 8
B_PER = B // N_CORES           # 2 items per core
DT = D // 128                  # 4 d-subtiles
NC128 = N // 128               # 32 n-chunks of 128
NC512 = N // 512               # 8 n-chunks of 512
TW = 520                       # padded row width of descT (512 + ones + pad)
WSCALE = 16.0                  # W pre-scale for fp8 range; exp scale undoes

_CACHE = {}


def _build():
    import concourse.bass as bass  # noqa: F401
    import concourse.tile as tile
    from concourse import bacc, mybir
    from contextlib import ExitStack

    bf16 = mybir.dt.bfloat16
    f32 = mybir.dt.float32
    f8 = mybir.dt.float8e4
    AF = mybir.ActivationFunctionType
    OP = mybir.AluOpType
    AX = mybir.AxisListType
    DR = mybir.MatmulPerfMode.DoubleRow

    nc = bacc.Bacc("TRN2", target_bir_lowering=False, debug=False,
                   num_devices=N_CORES)

    desc8_d = nc.dram_tensor("desc8", [B_PER, 8, 128, DT, N // 8], f8,
                             kind="ExternalInput").ap()
    descT_d = nc.dram_tensor("descT", [B_PER, 128, NC128, TW], f8,
                             kind="ExternalInput").ap()
    wt_d = nc.dram_tensor("wt", [128, DT, K], f8, kind="ExternalInput").ap()
    b_d = nc.dram_tensor("bias", [K, 1], f32, kind="ExternalInput").ap()
    cneg_d = nc.dram_tensor("cneg", [K, D], f32, kind="ExternalInput").ap()
    eyez_d = nc.dram_tensor("eyez", [K, K + 1], bf16, kind="ExternalInput").ap()
    out_d = nc.dram_tensor("out", [B_PER, K, D], f32, kind="ExternalOutput").ap()

    with tile.TileContext(nc) as tc, ExitStack() as ctx:
        const = ctx.enter_context(tc.tile_pool(name="const", bufs=1))
        big = ctx.enter_context(tc.tile_pool(name="big", bufs=2))
        bigT = ctx.enter_context(tc.tile_pool(name="bigT", bufs=2))
        med = ctx.enter_context(tc.tile_pool(name="med", bufs=2))
        small = ctx.enter_context(tc.tile_pool(name="small", bufs=4))
        ps_sc = ctx.enter_context(tc.tile_pool(name="ps_sc", bufs=2, space="PSUM"))
        ps_etz = ctx.enter_context(tc.tile_pool(name="ps_etz", bufs=4, space="PSUM"))
        ps_agg = ctx.enter_context(tc.tile_pool(name="ps_agg", bufs=1, space="PSUM"))

        NS = N // 8  # 512-col desc8 DMA slices == one mm1 chunk each

        # Queue plan (ring order == issue order; both rings split HBM bw):
        #   sync  q: d8_i0 x8, consts, T_i1 x4, cneg, out stores
        #   scalar q: T_i0 x4, d8_i1 x8
        # The globally last-arriving slices are T_i1's, whose remaining
        # dependency chain (2 matmuls + tail) is the shortest.
        def load_desc(i):
            desc_sb = big.tile([128, DT, N], f8, tag="desc")
            descT_sb = bigT.tile([128, NC128, TW], f8, tag="descT")
            d8_eng = nc.sync if i == 0 else nc.scalar
            t_eng = nc.scalar if i == 0 else nc.sync
            if i == 1:
                for s in range(4):
                    t_eng.dma_start(
                        out=descT_sb[:, 8 * s:8 * (s + 1), :],
                        in_=descT_d[i, :, 8 * s:8 * (s + 1), :])
            for s in range(8):
                d8_eng.dma_start(out=desc_sb[:, :, NS * s:NS * (s + 1)],
                                 in_=desc8_d[i, s])
                if i == 0 and s == 1:
                    # consts ride the sync ring right behind the first two
                    # desc slices: wt/b present before mm1 c0 finishes
                    nc.sync.dma_start(out=wt_sb[:], in_=wt_d[:])
                    nc.sync.dma_start(out=b_sb[:], in_=b_d[:])
                    nc.sync.dma_start(out=eyez_sb[:], in_=eyez_d[:])
                if i == 0 and s == 7:
                    nc.sync.dma_start(out=cneg_sb[:], in_=cneg_d[:])
            if i == 0:
                for s in range(4):
                    t_eng.dma_start(
                        out=descT_sb[:, 8 * s:8 * (s + 1), :],
                        in_=descT_d[i, :, 8 * s:8 * (s + 1), :])
            return desc_sb, descT_sb

        # ---- constants (tiles; DMAs issued inside load_desc ordering) ----
        wt_sb = const.tile([128, DT, K], f8, tag="wt")
        b_sb = const.tile([K, 1], f32, tag="b")
        cneg_sb = const.tile([K, D], f32, tag="cneg")
        eyez_sb = const.tile([K, K + 1], bf16, tag="eyez")
        eps_sb = const.tile([K, 1], f32, tag="eps")
        nc.vector.memset(eps_sb[:], 1e-24)

        tiles = [load_desc(i) for i in range(B_PER)]

        for i in range(B_PER):
            desc_sb, descT_sb = tiles[i]
            exp_s = med.tile([K, N], bf16, tag="exp_s")
            softT = med.tile([128, NC128, K], f8, tag="softT")

            # ---- mm1 (DoubleRow fp8) + exp ----
            for c8 in range(NC512):
                csl = slice(512 * c8, 512 * (c8 + 1))
                sc_ps = ps_sc.tile([K, 512], f32, tag="sc")
                for tp in range(DT // 2):
                    nc.tensor.matmul(
                        sc_ps[:], lhsT=wt_sb[:, 2 * tp:2 * tp + 2, :],
                        rhs=desc_sb[:, 2 * tp:2 * tp + 2, csl],
                        start=(tp == 0), stop=(tp == DT // 2 - 1),
                        perf_mode=DR,
                    )
                nc.scalar.activation(out=exp_s[:, csl], in_=sc_ps[:],
                                     func=AF.Exp, bias=b_sb[:],
                                     scale=1.0 / WSCALE)

            # ---- expT + Z (one matmul per 128-chunk), softmax -> fp8 ----
            for g in range(8):
                etz_ps = ps_etz.tile([128, 4, K + 1], f32, tag="etz")
                for j in range(4):
                    c = 4 * g + j
                    nc.tensor.matmul(
                        etz_ps[:, j, :], lhsT=exp_s[:, 128 * c:128 * (c + 1)],
                        rhs=eyez_sb[:], start=True, stop=True,
                    )
                rz = small.tile([128, 4, 1], f32, tag="rz")
                nc.vector.reciprocal(rz[:], etz_ps[:, :, K:K + 1])
                nc.vector.tensor_mul(
                    softT[:, 4 * g:4 * (g + 1), :],
                    etz_ps[:, :, 0:K],
                    rz[:].broadcast_to([128, 4, K]),
                )

            # ---- mm2 (DoubleRow fp8), ssum in col 512 of descT ----
            aggA = ps_agg.tile([K, 256], f32, tag="aggA")
            aggB = ps_agg.tile([K, 257], f32, tag="aggB")
            for p in range(NC128 // 2):
                lhsT = softT[:, 2 * p:2 * p + 2, :]
                nc.tensor.matmul(aggA[:], lhsT=lhsT,
                                 rhs=descT_sb[:, 2 * p:2 * p + 2, 0:256],
                                 start=(p == 0), stop=(p == NC128 // 2 - 1),
                                 perf_mode=DR)
                nc.tensor.matmul(aggB[:], lhsT=lhsT,
                                 rhs=descT_sb[:, 2 * p:2 * p + 2, 256:513],
                                 start=(p == 0), stop=(p == NC128 // 2 - 1),
                                 perf_mode=DR)

            # ---- tail: vlad + normalizations ----
            ssum_sb = small.tile([K, 1], f32, tag="ssum")
            nc.scalar.copy(ssum_sb[:], aggB[:, 256:257])
            vlad_sb = med.tile([K, D], f32, tag="vlad")
            nc.vector.scalar_tensor_tensor(
                vlad_sb[:, 0:256], in0=cneg_sb[:, 0:256], scalar=ssum_sb[:],
                in1=aggA[:], op0=OP.mult, op1=OP.add,
            )
            nc.vector.scalar_tensor_tensor(
                vlad_sb[:, 256:512], in0=cneg_sb[:, 256:512], scalar=ssum_sb[:],
                in1=aggB[:, 0:256], op0=OP.mult, op1=OP.add,
            )
            # Intra-norm over D. The global L2 norm over the flattened
            # [D*K] vector is exactly 8 (64 unit-norm columns), so fold the
            # 1/8 into the per-column scale: 1/sqrt(64*ss) = rn/8.
            # NOTE: tensor_tensor_reduce crashes TRN2 here (device
            # unrecoverable) -- ACT Square + accum_out instead.
            sq_sb = med.tile([K, D], f32, tag="sq")
            ss_sb = small.tile([K, 1], f32, tag="ss")
            nc.scalar.activation(out=sq_sb[:], in_=vlad_sb[:],
                                 func=AF.Square, accum_out=ss_sb[:])
            sn_sb = small.tile([K, 1], f32, tag="sn")
            nc.scalar.activation(sn_sb[:], ss_sb[:], func=AF.Sqrt,
                                 bias=eps_sb[:], scale=64.0)
            rn_sb = small.tile([K, 1], f32, tag="rn")
            nc.vector.reciprocal(rn_sb[:], sn_sb[:])
            outT_sb = med.tile([K, D], f32, tag="outT")
            nc.vector.tensor_scalar_mul(outT_sb[:, 0:256], vlad_sb[:, 0:256],
                                        rn_sb[:])
            nc.sync.dma_start(out=out_d[i, :, 0:256], in_=outT_sb[:, 0:256])
            nc.vector.tensor_scalar_mul(outT_sb[:, 256:512],
                                        vlad_sb[:, 256:512], rn_sb[:])
            nc.sync.dma_start(out=out_d[i, :, 256:512],
                              in_=outT_sb[:, 256:512])

    nc.compile()
    return nc


def _get_nc():
    if "nc" not in _CACHE:
        _CACHE["nc"] = _build()
    return _CACHE["nc"]


def _host_inputs(descriptors, W, b, centers):
    bf16 = ml_dtypes.bfloat16
    f8 = ml_dtypes.float8_e4m3

    # quantize desc ONCE; both device layouts view the same fp8 values
    desc8_full = descriptors.astype(np.float32).astype(f8)  # [B, D, N]

    wt16 = (WSCALE * W.astype(np.float32)).astype(f8)       # [K, D]
    wt = np.ascontiguousarray(
        wt16.T.reshape(DT, 128, K).transpose(1, 0, 2))      # [128, DT, K]
    bias = np.ascontiguousarray(b.astype(np.float32).reshape(K, 1))
    cneg = np.ascontiguousarray((-centers.astype(np.float32).T))
    eyez = np.concatenate(
        [np.eye(K, dtype=np.float32), np.ones((K, 1), np.float32)],
        axis=1).astype(bf16)                                # [K, K+1]
    common = {"wt": wt, "bias": bias, "cneg": cneg, "eyez": eyez}

    in_maps = []
    for core in range(N_CORES):
        m = dict(common)
        d8 = desc8_full[B_PER * core:B_PER * (core + 1)]    # [B_PER, D, N] fp8
        # mm1 layout: [B_PER, 8, 128, DT, N/8], (s,p,t,nn) = d8[i, 128t+p,
        # 512s+nn]; slice s is contiguous for the DMA
        m["desc8"] = np.ascontiguousarray(
            d8.reshape(B_PER, DT, 128, 8, N // 8).transpose(0, 3, 2, 1, 4))
        # mm2 layout: [B_PER, 128, NC128, TW], (p,c,j) = d8[i, j, 128c+p]
        dT = d8.transpose(0, 2, 1)                          # [B_PER, N, D]
        aug = np.zeros((B_PER, N, TW), dtype=f8)
        aug[:, :, :D] = dT
        aug[:, :, D] = np.float32(1.0)
        m["descT"] = np.ascontiguousarray(
            aug.reshape(B_PER, NC128, 128, TW).transpose(0, 2, 1, 3))
        in_maps.append(m)
    return in_maps


def _run(inputs, trace=False):
    from concourse.bass_utils import run_bass_kernel_spmd

    descriptors = np.asarray(inputs["descriptors"])
    W = np.asarray(inputs["W"])
    b = np.asarray(inputs["b"])
    centers = np.asarray(inputs["centers"])
    nc = _get_nc()
    in_maps = _host_inputs(descriptors, W, b, centers)
    res = run_bass_kernel_spmd(nc, in_maps, list(range(N_CORES)), trace=trace)
    outs = []
    for core in range(N_CORES):
        o = res.results[core]["out"]          # [B_PER, K, D]
        outs.append(np.transpose(o, (0, 2, 1)).reshape(B_PER, D * K))
    full = np.concatenate(outs, axis=0).astype(np.float32)
    return full, res


def kernel(**inputs):
    out, _ = _run(inputs, trace=False)
    return out


if __name__ == "__main__":
    rng = np.random.default_rng(0)
    inputs = {
        "descriptors": rng.standard_normal((B, D, N), dtype=np.float32),
        "W": (rng.standard_normal((K, D)) * 0.05).astype(np.float32),
        "b": (rng.standard_normal((K,)) * 0.05).astype(np.float32),
        "centers": rng.standard_normal((D, K)).astype(np.float32),
    }
    out = kernel(**inputs)
    print("out shape:", out.shape, out.dtype)


# revision 21
# speedup vs baseline: 1.2872x; 1.2872x over previous
"""NetVLAD layer on 8 Trainium2 NeuronCores (Bass/Tile), fp8 DoubleRow.

Problem: descriptors [B=16, D=512, N=4096] f32, W [K=64, D], b [K],
centers [D, K].
  scores = softmax_K(W @ desc + b)            [B, K, N]
  agg[b,d,k] = sum_n scores[b,k,n] desc[b,d,n]
  vlad = agg - centers * sum_n(scores);  intra-L2-norm over D; global L2.

Sharding: data-parallel over B across 8 cores (2 items per core);
W/b/centers replicated.

Host prep: desc quantized to fp8e4m3 ONCE, shipped in BOTH layouts:
  desc8  [B_PER, 128, 4, N]    (p,t,n) = q(desc[i, 128t+p, n])   - mm1 rhs
  descT  [B_PER, 128, 32, 520] (p,c,j) = q(desc[i, j, 128c+p]),
         col 512 = 1.0 (ssum rides mm2), 513..519 pad            - mm2 rhs
  wt     [128, 4, 64] = q(16*W[k, 128t+p])  (exp undoes the 16 via scale)

Per-core kernel (per item):
  - mm1 (fp8 DoubleRow, 2x rate): scores[64, 512-chunk] psum, 2 MMs/chunk
  - ACT: exp_s = Exp(scores/16 + b) -> bf16 SBUF [64, N]
  - expT+Z in one matmul per 128-chunk: lhsT=exp chunk [64,128],
    rhs=[eye64|ones] [64,65] -> psum [128n, 64k | Z]
  - DVE: rz = 1/Z (batched x4); softT = expT * rz -> fp8 (bcast mul)
  - mm2 (fp8 DoubleRow): agg[64k, 256|257] += softT_pair.T @ descT_pair
    col 512 of descT = ones => agg[:,512] = ssum
  - tail: vlad = cneg*ssum + agg; intra-norm over D (free dim);
    global norm via ones-matmul partition reduction; DMA out [64, 512]
Host side: concat over B, transpose [K, D] -> [D, K] flatten.
"""

import sys

sys.path.insert(0, "/opt/trn_rl_repo")

import numpy as np
import ml_dtypes

B, D, K, N = 16, 512, 64, 4096
N_CORES = 8
B_PER = B // N_CORES           # 2 items per core
DT = D // 128                  # 4 d-subtiles
NC128 = N // 128               # 32 n-chunks of 128
NC512 = N // 512               # 8 n-chunks of 512
TW = 520                       # padded row width of descT (512 + ones + pad)
WSCALE = 16.0                  # W pre-scale for fp8 range; exp scale undoes

_CACHE = {}


def _build():
    import concourse.bass as bass  # noqa: F401
    import concourse.tile as tile
    from concourse import bacc, mybir
    from contextlib import ExitStack

    bf16 = mybir.dt.bfloat16
    f32 = mybir.dt.float32
    f8 = mybir.dt.float8e4
    AF = mybir.ActivationFunctionType
    OP = mybir.AluOpType
    AX = mybir.AxisListType
    DR = mybir.MatmulPerfMode.DoubleRow

    nc = bacc.Bacc("TRN2", target_bir_lowering=False, debug=False,
                   num_devices=N_CORES)

    desc8_d = nc.dram_tensor("desc8", [B_PER, 8, 128, DT, N // 8], f8,
                             kind="ExternalInput").ap()
    descT_d = nc.dram_tensor("descT", [B_PER, 128, NC128, TW], f8,
                             kind="ExternalInput").ap()
    wt_d = nc.dram_tensor("wt", [128, DT, K], f8, kind="ExternalInput").ap()
    b_d = nc.dram_tensor("bias", [K, 1], f32, kind="ExternalInput").ap()
    cneg_d = nc.dram_tensor("cneg", [K, D], f32, kind="ExternalInput").ap()
    eyez_d = nc.dram_tensor("eyez", [K, K + 1], bf16, kind="ExternalInput").ap()
    out_d = nc.dram_tensor("out", [B_PER, K, D], f32, kind="ExternalOutput").ap()

    with tile.TileContext(nc) as tc, ExitStack() as ctx:
        const = ctx.enter_context(tc.tile_pool(name="const", bufs=1))
        big = ctx.enter_context(tc.tile_pool(name="big", bufs=2))
        bigT = ctx.enter_context(tc.tile_pool(name="bigT", bufs=2))
        med = ctx.enter_context(tc.tile_pool(name="med", bufs=2))
        small = ctx.enter_context(tc.tile_pool(name="small", bufs=4))
        ps_sc = ctx.enter_context(tc.tile_pool(name="ps_sc", bufs=2, space="PSUM"))
        ps_etz = ctx.enter_context(tc.tile_pool(name="ps_etz", bufs=4, space="PSUM"))
        ps_agg = ctx.enter_context(tc.tile_pool(name="ps_agg", bufs=1, space="PSUM"))

        NS = N // 8  # 512-col desc8 DMA slices == one mm1 chunk each

        # ---- constants (tiles; DMAs issued inside the load ordering) ----
        wt_sb = const.tile([128, DT, K], f8, tag="wt")
        b_sb = const.tile([K, 1], f32, tag="b")
        cneg_sb = const.tile([K, D], f32, tag="cneg")
        eyez_sb = const.tile([K, K + 1], bf16, tag="eyez")
        eps_sb = const.tile([K, 1], f32, tag="eps")
        nc.vector.memset(eps_sb[:], 1e-24)

        # Queue plan (ring order == issue order). Blocking DMA triggers are
        # only safe on the otherwise-idle sync engine; the scalar engine
        # gets few triggers so exp ACTs are never queued behind them.
        #   sync ring:  d8_i0 x8 (+consts), d8_i1 x8, T_i1 chunks 16..31,
        #               out stores
        #   scalar ring: T_i0 x4 (top), T_i1 chunks 0..15 (mid-compute)
        # Globally last-arriving: T_i1's tail chunks, whose remaining
        # dependency chain (2 mm2 matmuls + tail) is the shortest.
        tiles = []
        for i in range(B_PER):
            desc_sb = big.tile([128, DT, N], f8, tag="desc")
            descT_sb = bigT.tile([128, NC128, TW], f8, tag="descT")
            tiles.append((desc_sb, descT_sb))
            for s in range(8):
                nc.sync.dma_start(out=desc_sb[:, :, NS * s:NS * (s + 1)],
                                  in_=desc8_d[i, s])
                if i == 0 and s == 1:
                    nc.sync.dma_start(out=wt_sb[:], in_=wt_d[:])
                    nc.sync.dma_start(out=b_sb[:], in_=b_d[:])
                    nc.sync.dma_start(out=eyez_sb[:], in_=eyez_d[:])
                if i == 0 and s == 7:
                    nc.sync.dma_start(out=cneg_sb[:], in_=cneg_d[:])
            if i == 0:
                for s in range(4):
                    nc.scalar.dma_start(
                        out=descT_sb[:, 8 * s:8 * (s + 1), :],
                        in_=descT_d[i, :, 8 * s:8 * (s + 1), :])
        for s in range(2, 4):  # T_i1 late half on sync, after both d8s
            nc.sync.dma_start(out=tiles[1][1][:, 8 * s:8 * (s + 1), :],
                              in_=descT_d[1, :, 8 * s:8 * (s + 1), :])

        for i in range(B_PER):
            desc_sb, descT_sb = tiles[i]
            exp_s = med.tile([K, N], bf16, tag="exp_s")
            softT = med.tile([128, NC128, K], f8, tag="softT")

            # ---- mm1 (DoubleRow fp8) + exp ----
            for c8 in range(NC512):
                csl = slice(512 * c8, 512 * (c8 + 1))
                sc_ps = ps_sc.tile([K, 512], f32, tag="sc")
                for tp in range(DT // 2):
                    nc.tensor.matmul(
                        sc_ps[:], lhsT=wt_sb[:, 2 * tp:2 * tp + 2, :],
                        rhs=desc_sb[:, 2 * tp:2 * tp + 2, csl],
                        start=(tp == 0), stop=(tp == DT // 2 - 1),
                        perf_mode=DR,
                    )
                nc.scalar.activation(out=exp_s[:, csl], in_=sc_ps[:],
                                     func=AF.Exp, bias=b_sb[:],
                                     scale=1.0 / WSCALE)

            if i == 0:
                # T_i1 early half: 2 scalar triggers, issued only now so
                # item 0's exp ACTs were not queued behind them
                for s in range(0, 2):
                    nc.scalar.dma_start(
                        out=tiles[1][1][:, 8 * s:8 * (s + 1), :],
                        in_=descT_d[1, :, 8 * s:8 * (s + 1), :])

            # ---- expT + Z (one matmul per 128-chunk), softmax -> fp8 ----
            for g in range(8):
                etz_ps = ps_etz.tile([128, 4, K + 1], f32, tag="etz")
                for j in range(4):
                    c = 4 * g + j
                    nc.tensor.matmul(
                        etz_ps[:, j, :], lhsT=exp_s[:, 128 * c:128 * (c + 1)],
                        rhs=eyez_sb[:], start=True, stop=True,
                    )
                rz = small.tile([128, 4, 1], f32, tag="rz")
                nc.vector.reciprocal(rz[:], etz_ps[:, :, K:K + 1])
                nc.vector.tensor_mul(
                    softT[:, 4 * g:4 * (g + 1), :],
                    etz_ps[:, :, 0:K],
                    rz[:].broadcast_to([128, 4, K]),
                )

            # ---- mm2 (DoubleRow fp8), ssum in col 512 of descT ----
            aggA = ps_agg.tile([K, 256], f32, tag="aggA")
            aggB = ps_agg.tile([K, 257], f32, tag="aggB")
            for p in range(NC128 // 2):
                lhsT = softT[:, 2 * p:2 * p + 2, :]
                nc.tensor.matmul(aggA[:], lhsT=lhsT,
                                 rhs=descT_sb[:, 2 * p:2 * p + 2, 0:256],
                                 start=(p == 0), stop=(p == NC128 // 2 - 1),
                                 perf_mode=DR)
                nc.tensor.matmul(aggB[:], lhsT=lhsT,
                                 rhs=descT_sb[:, 2 * p:2 * p + 2, 256:513],
                                 start=(p == 0), stop=(p == NC128 // 2 - 1),
                                 perf_mode=DR)

            # ---- tail: vlad + normalizations ----
            ssum_sb = small.tile([K, 1], f32, tag="ssum")
            nc.scalar.copy(ssum_sb[:], aggB[:, 256:257])
            vlad_sb = med.tile([K, D], f32, tag="vlad")
            nc.vector.scalar_tensor_tensor(
                vlad_sb[:, 0:256], in0=cneg_sb[:, 0:256], scalar=ssum_sb[:],
                in1=aggA[:], op0=OP.mult, op1=OP.add,
            )
            nc.vector.scalar_tensor_tensor(
                vlad_sb[:, 256:512], in0=cneg_sb[:, 256:512], scalar=ssum_sb[:],
                in1=aggB[:, 0:256], op0=OP.mult, op1=OP.add,
            )
            # Intra-norm over D. The global L2 norm over the flattened
            # [D*K] vector is exactly 8 (64 unit-norm columns), so fold the
            # 1/8 into the per-column scale: 1/sqrt(64*ss) = rn/8.
            # NOTE: tensor_tensor_reduce crashes TRN2 here (device
            # unrecoverable) -- ACT Square + accum_out instead.
            sq_sb = med.tile([K, D], f32, tag="sq")
            ss_sb = small.tile([K, 1], f32, tag="ss")
            nc.scalar.activation(out=sq_sb[:], in_=vlad_sb[:],
                                 func=AF.Square, accum_out=ss_sb[:])
            sn_sb = small.tile([K, 1], f32, tag="sn")
            nc.scalar.activation(sn_sb[:], ss_sb[:], func=AF.Sqrt,
                                 bias=eps_sb[:], scale=64.0)
            rn_sb = small.tile([K, 1], f32, tag="rn")
            nc.vector.reciprocal(rn_sb[:], sn_sb[:])
            outT_sb = med.tile([K, D], f32, tag="outT")
            nc.vector.tensor_scalar_mul(outT_sb[:, 0:256], vlad_sb[:, 0:256],
                                        rn_sb[:])
            nc.sync.dma_start(out=out_d[i, :, 0:256], in_=outT_sb[:, 0:256])
            nc.vector.tensor_scalar_mul(outT_sb[:, 256:512],
                                        vlad_sb[:, 256:512], rn_sb[:])
            nc.sync.dma_start(out=out_d[i, :, 256:512],
                              in_=outT_sb[:, 256:512])

    nc.compile()
    return nc


def _get_nc():
    if "nc" not in _CACHE:
        _CACHE["nc"] = _build()
    return _CACHE["nc"]


def _host_inputs(descriptors, W, b, centers):
    bf16 = ml_dtypes.bfloat16
    f8 = ml_dtypes.float8_e4m3

    # quantize desc ONCE; both device layouts view the same fp8 values
    desc8_full = descriptors.astype(np.float32).astype(f8)  # [B, D, N]

    wt16 = (WSCALE * W.astype(np.float32)).astype(f8)       # [K, D]
    wt = np.ascontiguousarray(
        wt16.T.reshape(DT, 128, K).transpose(1, 0, 2))      # [128, DT, K]
    bias = np.ascontiguousarray(b.astype(np.float32).reshape(K, 1))
    cneg = np.ascontiguousarray((-centers.astype(np.float32).T))
    eyez = np.concatenate(
        [np.eye(K, dtype=np.float32), np.ones((K, 1), np.float32)],
        axis=1).astype(bf16)                                # [K, K+1]
    common = {"wt": wt, "bias": bias, "cneg": cneg, "eyez": eyez}

    in_maps = []
    for core in range(N_CORES):
        m = dict(common)
        d8 = desc8_full[B_PER * core:B_PER * (core + 1)]    # [B_PER, D, N] fp8
        # mm1 layout: [B_PER, 8, 128, DT, N/8], (s,p,t,nn) = d8[i, 128t+p,
        # 512s+nn]; slice s is contiguous for the DMA
        m["desc8"] = np.ascontiguousarray(
            d8.reshape(B_PER, DT, 128, 8, N // 8).transpose(0, 3, 2, 1, 4))
        # mm2 layout: [B_PER, 128, NC128, TW], (p,c,j) = d8[i, j, 128c+p]
        dT = d8.transpose(0, 2, 1)                          # [B_PER, N, D]
        aug = np.zeros((B_PER, N, TW), dtype=f8)
        aug[:, :, :D] = dT
        aug[:, :, D] = np.float32(1.0)
        m["descT"] = np.ascontiguousarray(
            aug.reshape(B_PER, NC128, 128, TW).transpose(0, 2, 1, 3))
        in_maps.append(m)
    return in_maps


def _run(inputs, trace=False):
    from concourse.bass_utils import run_bass_kernel_spmd

    descriptors = np.asarray(inputs["descriptors"])
    W = np.asarray(inputs["W"])
    b = np.asarray(inputs["b"])
    centers = np.asarray(inputs["centers"])
    nc = _get_nc()
    in_maps = _host_inputs(descriptors, W, b, centers)
    res = run_bass_kernel_spmd(nc, in_maps, list(range(N_CORES)), trace=trace)
    outs = []
    for core in range(N_CORES):
        o = res.results[core]["out"]          # [B_PER, K, D]
        outs.append(np.transpose(o, (0, 2, 1)).reshape(B_PER, D * K))
    full = np.concatenate(outs, axis=0).astype(np.float32)
    return full, res


def kernel(**inputs):
    out, _ = _run(inputs, trace=False)
    return out


if __name__ == "__main__":
    rng = np.random.default_rng(0)
    inputs = {
        "descriptors": rng.standard_normal((B, D, N), dtype=np.float32),
        "W": (rng.standard_normal((K, D)) * 0.05).astype(np.float32),
        "b": (rng.standard_normal((K,)) * 0.05).astype(np.float32),
        "centers": rng.standard_normal((D, K)).astype(np.float32),
    }
    out = kernel(**inputs)
    print("out shape:", out.shape, out.dtype)
